# revision 47
# baseline (speedup 1.0000x reference)
"""Trainium2 Bass kernel for nn_Mlp_cnn_shift (dense CNN MLP with 3x3 patch-shift
and a softmax-gated mix of two branches).

Strategy
--------
Data-parallel over the 16 (B,T) frames: each of the 8 NeuronCores processes 2
frames end-to-end.  All activations are kept channel-major ([C, tokens]) so the
channel contraction of every matmul has K on partitions, and `x` is
pre-transposed/cast on the host so no on-device transpose is needed.

Patch-shift handling:
 * forward shift (on xh, HID=1024): xh is stored in a zero-padded token layout
   (row pitch 57 = 56 cols + 1 zero pad col, 58-token zero guards per frame)
   and in 9 channel groups of 114 padded to 128 partitions each (host-permuted
   fc_w columns / fc1_w+fc2_w rows).  Every (dh,dw) roll then becomes a pure
   token offset in the fc1 matmul's rhs access pattern, with the zero padding
   reproducing the reference's zero-fill boundary exactly.
 * inverse shift (on gelu(y), C=512): y's channels live in an INTERLEAVED-512
   row permutation: the 8 main shift groups (57 channels each) sit at rows
   64*g .. 64*g+57, and the 9th group's 56 channels exactly fill the 7 spare
   rows of each 64-row half (rows 64*hb+57 .. 64*hb+64).  512 = 4 dense
   128-row blocks, no pad rows at all — fc1/fc2/proj all contract over
   exactly 4 K-blocks.  The gelu PSUM evacuation applies the inverse shift by
   writing each group's rows at its shifted, edge-clipped token positions
   (two 57-row writes plus two 7-row g8 fragments per block; the hardware
   accepts non-32-aligned partition bases for ScalarE/DVE access patterns).
   The w branch needs no shift, so its evacuation stays one full-block write.

Instead of w, the kernel spills d = h - w (computed per row-group on the DVE,
one row-group behind the evacuations); the post-gate combine is then just
gated = h + (a0-1)*d, two cheap DVE ops per chunk.

The only cross-core coupling is the global (T,H,W) mean feeding the softmax
gate, done as per-batch subgroup AllReduces ([0-3] and [4-7]), one per frame:
the first overlaps frame-1 compute; the second's window (collective latency
plus cross-core launch skew) is bridged by a budget of warm matmuls that
keeps the PE activity monitor from re-throttling the clock.

bf16 matmuls with f32 PSUM accumulation; output stored bf16, upcast on host.
"""

import os
import sys

for _p in ("/opt/trn_rl_repo",):
    if os.path.isdir(_p) and _p not in sys.path:
        sys.path.append(_p)

import numpy as np
import ml_dtypes

import concourse.bass as bass  # noqa: F401
import concourse.mybir as mybir
import concourse.tile as tile
from concourse import bacc
from concourse.bass_utils import run_bass_kernel_spmd

# ---------------------------------------------------------------- constants
SHIFTS = [(1, 1), (1, 0), (1, -1), (0, 1), (0, 0), (0, -1), (-1, 1), (-1, 0), (-1, -1)]
NG = 9
B, T, H, W, C = 2, 8, 56, 56, 512
HID = 1024
NCORES = 8
NF = (B * T) // NCORES          # frames per core = 2
HWTOK = H * W                   # 3136 tokens per frame
RP = W + 1                      # padded row pitch = 57
GUARD = RP + 1                  # 58 zero tokens on each end
FRPAD = RP * H                  # 3192
XHSPAN = GUARD + FRPAD + GUARD  # 3308
RG = 7                          # row groups per frame
RGR = H // RG                   # 8 rows per group
RGT = RGR * W                   # 448 valid tokens per row group
RGP = RGR * RP                  # 456 padded tokens per row group
GS_HID = 114                    # hid shift-group size (9*114 = 1026 >= 1024)
GS_C = 57                       # C shift-group size (8 main groups)
G8N = C - 8 * GS_C              # 56 channels in the 9th group
CB = C // 128                   # 4 dense C row-blocks (interleaved layout)
CCB = C // 128                  # 4
MEAN_N = float(T * H * W)
WARM_MMS = 130                  # AllReduce-window bridge matmuls
HEAD_MMS = 20                   # kernel-head HAM warmup matmuls
DCK = 256                       # output-phase token chunk

F32 = mybir.dt.float32
BF16 = mybir.dt.bfloat16
BF16_NP = ml_dtypes.bfloat16

_CACHE = {}


def _qof():
    """Interleaved-512 layout: padded row q -> real channel (a permutation).

    Main group g (57 ch) at rows 64g..64g+57; g8 channel i at row
    64*(i//7) + 57 + (i%7).
    """
    q = np.full((C,), -1, np.int64)
    for g in range(8):
        q[64 * g:64 * g + GS_C] = np.arange(GS_C * g, GS_C * (g + 1))
    for i in range(G8N):
        q[64 * (i // 7) + GS_C + (i % 7)] = 8 * GS_C + i
    assert (np.sort(q) == np.arange(C)).all()
    return q


# ---------------------------------------------------------------- device kernel
def build_nc():
    nc = bacc.Bacc("TRN2", target_bir_lowering=False, debug=False, num_devices=NCORES)

    dp = nc.declare_dram_parameter
    xT = dp("xT", [NF, 128, CCB, HWTOK], BF16, isOutput=False)
    fcw = dp("fcw", [128, CCB, NG * 128], BF16, isOutput=False)
    fcb = dp("fcb", [128, NG], F32, isOutput=False)
    fc1w = dp("fc1w", [128, NG, C], BF16, isOutput=False)
    fc1b = dp("fc1b", [128, CB], F32, isOutput=False)
    fc2w = dp("fc2w", [128, NG, C], BF16, isOutput=False)
    fc2b = dp("fc2b", [128, CB], F32, isOutput=False)
    projw = dp("projw", [128, CB, C], BF16, isOutput=False)
    projb = dp("projb", [128, C], F32, isOutput=False)
    rw1w = dp("rw1w", [128, CB, 128], BF16, isOutput=False)
    rw1b = dp("rw1b", [128, 1], F32, isOutput=False)
    rw2w = dp("rw2w", [128, 2 * C], BF16, isOutput=False)
    rw2b = dp("rw2b", [128, 2 * CB], F32, isOutput=False)
    fmask = dp("fmask", [128, CB], F32, isOutput=False)
    out_d = dp("out", [NF, HWTOK, C], BF16, isOutput=True)

    # spill space for d = h - w of each frame + collective bounce buffers
    dsp = [nc.dram_tensor(f"dsp{f}", [128, CB, HWTOK], BF16) for f in range(NF)]
    ccin = [nc.dram_tensor(f"ccin{f}", [128, CB], F32) for f in range(NF)]
    ccout = [nc.dram_tensor(f"ccout{f}", [128, CB], F32) for f in range(NF)]

    AF = mybir.ActivationFunctionType
    ALU = mybir.AluOpType
    GROUPS = [list(range(NCORES // 2)), list(range(NCORES // 2, NCORES))]

    with tile.TileContext(nc, num_cores=NCORES) as tc:
        with (
            tc.tile_pool(name="singles", bufs=1) as singles,
            tc.tile_pool(name="xh_pool", bufs=1) as xh_pool,
            tc.tile_pool(name="h_pool", bufs=2) as h_pool,
            tc.tile_pool(name="w_pool", bufs=3) as w_pool,
            tc.tile_pool(name="xt_pool", bufs=2) as xt_pool,
            tc.tile_pool(name="ostage", bufs=8) as ostage,
            tc.tile_pool(name="dstream", bufs=6) as dstream,
            tc.tile_pool(name="small", bufs=1) as small,
            tc.tile_pool(name="mmpsum", bufs=8, space="PSUM") as mmpsum,
        ):
            # ---- HAM warm-up: junk matmuls on a scratch tile, no DMA deps,
            # so the PE is already at full clock when the first real matmul
            # arrives (results discarded).
            jt = singles.tile([128, 640], BF16, name="jt")
            nc.vector.memset(jt[:], 0.0)
            for wi in range(HEAD_MMS):
                wp = mmpsum.tile([128, 512], F32, tag="mm", name=f"hw{wi}")
                nc.tensor.matmul(
                    wp[:, :512], lhsT=jt[:, 0:128], rhs=jt[:, 128:640],
                    start=True, stop=True,
                )

            # ---- load weights (resident for the whole kernel)
            def load(name, shape, dtype, src):
                t = singles.tile(shape, dtype, name=name)
                nc.sync.dma_start(out=t, in_=src[:])
                return t

            # fc weights load per k-slice so the first matmul can start after
            # ~1/4 of the transfer; the rest of the weights load while frame
            # 0's fc pass runs.
            fcw_s = singles.tile([128, CCB, NG * 128], BF16, name="fcw_s")
            for k in range(CCB):
                nc.sync.dma_start(out=fcw_s[:, k, :], in_=fcw[:, k, :])
            fcb_s = load("fcb_s", [128, NG], F32, fcb)
            _rest = {}

            def load_rest():
                _rest["fc1w_s"] = load("fc1w_s", [128, NG, C], BF16, fc1w)
                _rest["fc1b_s"] = load("fc1b_s", [128, CB], F32, fc1b)
                _rest["fc2w_s"] = load("fc2w_s", [128, NG, C], BF16, fc2w)
                _rest["fc2b_s"] = load("fc2b_s", [128, CB], F32, fc2b)
                _rest["projw_s"] = load("projw_s", [128, CB, C], BF16, projw)
                _rest["projb_s"] = load("projb_s", [128, C], F32, projb)
                _rest["rw1w_s"] = load("rw1w_s", [128, CB, 128], BF16, rw1w)
                _rest["rw1b_s"] = load("rw1b_s", [128, 1], F32, rw1b)
                _rest["rw2w_s"] = load("rw2w_s", [128, 2 * C], BF16, rw2w)
                _rest["rw2b_s"] = load("rw2b_s", [128, 2 * CB], F32, rw2b)
                _rest["fm_s"] = load("fm_s", [128, CB], F32, fmask)

            a0_s = singles.tile([128, CB], F32)   # gate for the h branch
            a1_s = singles.tile([128, CB], F32)   # a0 - 1

            # xh, padded token layout, persistent across frames.
            xh = xh_pool.tile([128, NG, XHSPAN], BF16)
            nc.vector.memset(xh[:, :, :GUARD], 0.0)
            nc.vector.memset(xh[:, :, GUARD + FRPAD:], 0.0)
            xh_rows = xh[:, :, GUARD:GUARD + FRPAD].rearrange(
                "p g (r c) -> p g r c", c=RP
            )
            nc.vector.memset(xh_rows[:, :, :, W:], 0.0)

            hw_tiles = []

            def shifted_rhs(g, rg):
                off = -(SHIFTS[g][0] * RP + SHIFTS[g][1])
                s0 = GUARD + rg * RGP + off
                return xh[:, g, s0:s0 + RGP].rearrange(
                    "p (r c) -> p r c", c=RP
                )[:, :, :W]

            for f in range(NF):
                # ---------------- A: xh = gelu(x @ fc_w + fc_b), group-blocked
                for rg in range(RG):
                    xt_t = xt_pool.tile([128, CCB, RGT], BF16, tag="xt")
                    nc.sync.dma_start(
                        out=xt_t, in_=xT[f, :, :, rg * RGT:(rg + 1) * RGT]
                    )
                    for mb in range(NG):
                        ps = mmpsum.tile([128, 512], F32, tag="mm")
                        for k in range(CCB):
                            nc.tensor.matmul(
                                ps[:, :RGT],
                                lhsT=fcw_s[:, k, mb * 128:(mb + 1) * 128],
                                rhs=xt_t[:, k, :],
                                start=(k == 0),
                                stop=(k == CCB - 1),
                            )
                        dst = xh[
                            :, mb, GUARD + rg * RGP:GUARD + (rg + 1) * RGP
                        ].rearrange("p (r c) -> p r c", c=RP)[:, :, :W]
                        src = ps[:, :RGT].rearrange("p (r c) -> p r c", c=W)
                        nc.scalar.activation(
                            out=dst, in_=src, func=AF.Gelu,
                            bias=fcb_s[:, mb:mb + 1],
                        )

                if f == 0:
                    # frame-0 fc pass is in flight; now bring in the rest
                    load_rest()
                    fc1w_s = _rest["fc1w_s"]; fc1b_s = _rest["fc1b_s"]
                    fc2w_s = _rest["fc2w_s"]; fc2b_s = _rest["fc2b_s"]
                    projw_s = _rest["projw_s"]; projb_s = _rest["projb_s"]
                    rw1w_s = _rest["rw1w_s"]; rw1b_s = _rest["rw1b_s"]
                    rw2w_s = _rest["rw2w_s"]; rw2b_s = _rest["rw2b_s"]
                    fm_s = _rest["fm_s"]

                # ---------------- C: h = invshift(gelu(shift(xh) @ fc1_w + b))
                h_t = h_pool.tile([128, CB, HWTOK], BF16, tag="h")
                nc.gpsimd.memset(h_t[:], 0.0)
                h4 = h_t.rearrange("p c (i j) -> p c i j", j=W)
                wsum_st = small.tile([128, CB, RG], F32, tag=f"wsst{f}")

                def h_evac(ps, rg, mb):
                    """Inverse-shift evacuation of one 128-row block: two
                    57-row main-group writes, plus the two 7-row g8 fragments
                    via a 32-aligned scratch activation (compute-engine APs
                    must start at 32-aligned partitions) whose g8 rows are
                    then DMA-copied into place.  No accum_out — the gate sum
                    comes from one DVE reduce over the finished h tile."""
                    ps3 = ps[:, :RGT].rearrange("p (r c) -> p r c", c=W)
                    for half in range(2):
                        p0 = half * 64
                        g = 2 * mb + half
                        sh, sw = SHIFTS[g]
                        i0 = max(0, 8 * rg - sh)
                        i1 = min(H, 8 * rg + 8 - sh)
                        j0, j1 = max(0, -sw), min(W, W - sw)
                        nc.scalar.activation(
                            out=h4[p0:p0 + GS_C, mb, i0:i1, j0:j1],
                            in_=ps3[
                                p0:p0 + GS_C,
                                i0 + sh - 8 * rg:i1 + sh - 8 * rg,
                                j0 + sw:j1 + sw,
                            ],
                            func=AF.Gelu,
                            bias=fc1b_s[p0:p0 + GS_C, mb:mb + 1],
                        )
                    # g8 fragments via 32-row scratch activations (a pattern
                    # starting at partition 32k may span at most 32
                    # partitions, so one per half)
                    sh, sw = SHIFTS[8]
                    i0 = max(0, 8 * rg - sh)
                    i1 = min(H, 8 * rg + 8 - sh)
                    j0, j1 = max(0, -sw), min(W, W - sw)
                    scr = w_pool.tile([128, RGR + 1, W], BF16, tag="scr")
                    for half in range(2):
                        q0 = half * 64 + 32
                        nc.scalar.activation(
                            out=scr[q0:q0 + 32, 0:i1 - i0, j0:j1],
                            in_=ps3[
                                q0:q0 + 32,
                                i0 + sh - 8 * rg:i1 + sh - 8 * rg,
                                j0 + sw:j1 + sw,
                            ],
                            func=AF.Gelu,
                            bias=fc1b_s[q0:q0 + 32, mb:mb + 1],
                        )
                        f0 = half * 64 + GS_C
                        nc.sync.dma_start(
                            out=h4[f0:f0 + 7, mb, i0:i1, j0:j1],
                            in_=scr[f0:f0 + 7, 0:i1 - i0, j0:j1],
                        )

                # ---------------- C+B merged per row-group: fc1 (h, shifted
                # evacuation) and fc2 (w) matmuls interleaved so the ScalarE
                # evacuation load stays well under the PE matmul rate; then
                # d = h - w one row-group behind (spilled for the out phase).
                dsum_st = small.tile([128, CB, RG], F32, tag=f"dsst{f}")
                prev_w = [None] * RG

                def d_spill(rg):
                    w_prev = prev_w[rg]
                    nc.vector.tensor_tensor(
                        w_prev[:],
                        h_t[:, :, rg * RGT:(rg + 1) * RGT],
                        w_prev[:],
                        ALU.subtract,
                    )
                    nc.sync.dma_start(
                        out=dsp[f][:, :, rg * RGT:(rg + 1) * RGT], in_=w_prev[:]
                    )
                    # gate-sum bookkeeping: sum(h) = sum(d) + sum(w)
                    nc.vector.tensor_reduce(
                        out=dsum_st[:, :, rg:rg + 1], in_=w_prev[:],
                        axis=mybir.AxisListType.X, op=ALU.add,
                    )

                for rg in range(RG):
                    w_rg = w_pool.tile([128, CB, RGT], BF16, tag="wrg")
                    rhs_pl = xh[:, :, GUARD + rg * RGP:GUARD + (rg + 1) * RGP
                                ].rearrange("p g (r c) -> p g r c", c=RP)[:, :, :, :W]
                    for mb in range(CB):
                        ps = mmpsum.tile([128, 512], F32, tag="mm")
                        for g in range(NG):
                            nc.tensor.matmul(
                                ps[:, :RGT],
                                lhsT=fc1w_s[:, g, mb * 128:(mb + 1) * 128],
                                rhs=shifted_rhs(g, rg),
                                start=(g == 0),
                                stop=(g == NG - 1),
                            )
                        h_evac(ps, rg, mb)
                        ps = mmpsum.tile([128, 512], F32, tag="mm")
                        for g in range(NG):
                            nc.tensor.matmul(
                                ps[:, :RGT],
                                lhsT=fc2w_s[:, g, mb * 128:(mb + 1) * 128],
                                rhs=rhs_pl[:, g],
                                start=(g == 0),
                                stop=(g == NG - 1),
                            )
                        dst = w_rg[:, mb, :].rearrange("p (r c) -> p r c", c=W)
                        srcp = ps[:, :RGT].rearrange("p (r c) -> p r c", c=W)
                        nc.scalar.activation(
                            out=dst, in_=srcp, func=AF.Gelu,
                            bias=fc2b_s[:, mb:mb + 1],
                        )
                    nc.vector.tensor_reduce(
                        out=wsum_st[:, :, rg:rg + 1], in_=w_rg[:],
                        axis=mybir.AxisListType.X, op=ALU.add,
                    )

                    prev_w[rg] = w_rg
                    if rg >= 1:
                        d_spill(rg - 1)
                d_spill(RG - 1)

                # ---------------- per-frame gate partial sums + AllReduce
                # part = sum(h) + sum(w) = sum(d) + 2*sum(w)
                ws = small.tile([128, CB], F32, tag=f"ws{f}")
                nc.vector.tensor_reduce(
                    out=ws, in_=wsum_st[:], axis=mybir.AxisListType.X, op=ALU.add
                )
                dsm = small.tile([128, CB], F32, tag=f"dsm{f}")
                nc.vector.tensor_reduce(
                    out=dsm, in_=dsum_st[:], axis=mybir.AxisListType.X, op=ALU.add
                )
                part = small.tile([128, CB], F32, tag=f"part{f}")
                nc.vector.tensor_scalar_mul(part, ws, 2.0)
                nc.vector.tensor_tensor(part, part, dsm, ALU.add)
                nc.sync.dma_start(out=ccin[f][:], in_=part)
                nc.gpsimd.collective_compute(
                    "AllReduce",
                    ALU.add,
                    replica_groups=GROUPS,
                    ins=[ccin[f][:]],
                    outs=[ccout[f][:]],
                )

                hw_tiles.append(h_t)

            # bridge the second AllReduce's latency window (collective floor
            # + cross-core launch skew) with junk matmuls so the PE activity
            # monitor keeps the full clock.
            for wi in range(WARM_MMS):
                wp = mmpsum.tile([128, 512], F32, tag="mm", name=f"warm{wi}")
                nc.tensor.matmul(
                    wp[:, :512],
                    lhsT=fcw_s[:, 0, 0:128],
                    rhs=fcw_s[:, 1, 0:512],
                    start=True,
                    stop=True,
                )

            # ---------------- combine the two AllReduce results -> z
            acc = []
            for f in range(NF):
                za = small.tile([128, CB], F32, tag=f"za{f}")
                nc.sync.dma_start(out=za, in_=ccout[f][:])
                acc.append(za)
            zsum = small.tile([128, CB], F32, tag="zsum")
            nc.vector.tensor_tensor(zsum, acc[0], acc[1], ALU.add)
            zbf = small.tile([128, CB], BF16, tag="zbf")
            nc.vector.tensor_copy(out=zbf, in_=zsum)

            # ---------------- gate: a = softmax over the 2 streams
            # (1/MEAN_N is folded into rw1w on the host)
            psg = mmpsum.tile([128, 512], F32, tag="mm", name="psg")[:, :1]
            for k in range(CB):
                nc.tensor.matmul(
                    psg,
                    lhsT=rw1w_s[:, k, :],
                    rhs=zbf[:, k:k + 1],
                    start=(k == 0),
                    stop=(k == CB - 1),
                )
            gv = small.tile([128, 1], BF16, tag="gv")
            nc.scalar.activation(out=gv, in_=psg, func=AF.Gelu, bias=rw1b_s[:, 0:1])
            psu = mmpsum.tile([128, 512], F32, tag="mm", name="psu")[:, :2 * CB]
            for m in range(2 * CB):
                nc.tensor.matmul(
                    psu[:, m:m + 1],
                    lhsT=rw2w_s[:, m * 128:(m + 1) * 128],
                    rhs=gv,
                    start=True,
                    stop=True,
                )
            # softmax over 2 streams == sigmoid of the logit difference:
            # a0 = e0/(e0+e1) = sigmoid(l0 - l1), a1 = a0 - 1
            uv = small.tile([128, 2 * CB], F32, tag="uv")
            nc.vector.tensor_tensor(uv, psu, rw2b_s, ALU.add)
            ldif = small.tile([128, CB], F32, tag="ldif")
            nc.vector.tensor_tensor(ldif, uv[:, 0:CB], uv[:, CB:2 * CB],
                                    ALU.subtract)
            nc.scalar.activation(out=a0_s, in_=ldif, func=AF.Sigmoid)
            nc.vector.tensor_scalar_sub(a1_s, a0_s, 1.0)

            # ---------------- D: out = (h + (a0-1)*d) @ proj_w + proj_b
            def emit_out(pp, fidx, t0, M):
                ot = ostage.tile([128, C], BF16, tag="ot")
                nc.vector.tensor_tensor(ot[:M], pp[:M, :C], projb_s[:M], ALU.add)
                nc.sync.dma_start(out=out_d[fidx, t0:t0 + M, :], in_=ot[:M])

            # h is resident for both frames; stream each frame's d back in
            # DCK-token chunks, gate into the d tile, then project.  The two
            # frames' chunks are interleaved so their independent
            # DMA->gate->matmul chains hide each other's latency.
            for ck0 in range(0, HWTOK, DCK):
                for fidx in (0, 1):
                    h_t = hw_tiles[fidx]
                    CK = min(DCK, HWTOK - ck0)
                    dc = dstream.tile([128, CB, DCK], BF16, tag="wc")
                    nc.sync.dma_start(
                        out=dc[:, :, :CK], in_=dsp[fidx][:, :, ck0:ck0 + CK]
                    )
                    # gating scale split across ScalarE and DVE so neither
                    # paces the chunk pipeline
                    for kb in range(2):
                        nc.scalar.activation(
                            out=dc[:, kb, :CK], in_=dc[:, kb, :CK],
                            func=AF.Copy, scale=a1_s[:, kb:kb + 1],
                        )
                    for kb in range(2, CB):
                        nc.vector.tensor_scalar_mul(
                            dc[:, kb, :CK], dc[:, kb, :CK], a1_s[:, kb:kb + 1]
                        )
                    nc.vector.tensor_tensor(
                        dc[:, :, :CK],
                        h_t[:, :, ck0:ck0 + CK],
                        dc[:, :, :CK],
                        ALU.add,
                    )
                    m0 = 0
                    while m0 < CK:
                        M = min(128, CK - m0)
                        pp = mmpsum.tile([128, 512], F32, tag="mm")
                        for kb in range(CB):
                            nc.tensor.matmul(
                                pp[:M, :C],
                                lhsT=dc[:, kb, m0:m0 + M],
                                rhs=projw_s[:, kb, :],
                                start=(kb == 0),
                                stop=(kb == CB - 1),
                            )
                        emit_out(pp, fidx, ck0 + m0, M)
                        m0 += M

    nc.compile()
    return nc


# ---------------------------------------------------------------- host side
def _prep_weights(fc_w, fc_b, fc1_w, fc1_b, fc2_w, fc2_b,
                  rw1_w, rw1_b, rw2_w, rw2_b, proj_w, proj_b):
    f32 = np.float32
    qof = _qof()

    # fc: columns permuted into 9 HID-groups of 114 (112 for g=8), pad to 128
    fcwp = np.zeros((C, NG * 128), f32)
    fcbp = np.zeros((NG * 128,), f32)
    for g in range(NG):
        n = min(GS_HID * (g + 1), HID) - GS_HID * g
        fcwp[:, 128 * g:128 * g + n] = fc_w[:, GS_HID * g:GS_HID * g + n]
        fcbp[128 * g:128 * g + n] = fc_b[GS_HID * g:GS_HID * g + n]
    fcw_h = np.ascontiguousarray(
        fcwp.reshape(CCB, 128, NG * 128).transpose(1, 0, 2)
    ).astype(BF16_NP)
    fcb_h = np.ascontiguousarray(fcbp.reshape(NG, 128).T).astype(f32)

    def hid_rows_grouped(wm):  # [HID, C] -> [128, NG, C] padded group rows
        wp = np.zeros((NG * 128, wm.shape[1]), f32)
        for g in range(NG):
            n = min(GS_HID * (g + 1), HID) - GS_HID * g
            wp[128 * g:128 * g + n] = wm[GS_HID * g:GS_HID * g + n]
        return np.ascontiguousarray(
            wp.reshape(NG, 128, wm.shape[1]).transpose(1, 0, 2)
        ).astype(BF16_NP)

    # fc1/fc2 columns, proj/rw1 rows, biases: interleaved-512 permutation
    fc1w_h = hid_rows_grouped(fc1_w[:, qof])
    fc2w_h = hid_rows_grouped(fc2_w[:, qof])
    fc1b_h = np.ascontiguousarray(fc1_b[qof].reshape(CB, 128).T).astype(f32)
    fc2b_h = np.ascontiguousarray(fc2_b[qof].reshape(CB, 128).T).astype(f32)

    projw_h = np.ascontiguousarray(
        proj_w[qof].reshape(CB, 128, C).transpose(1, 0, 2)
    ).astype(BF16_NP)
    projb_h = np.ascontiguousarray(
        np.broadcast_to(proj_b[None, :], (128, C))
    ).astype(f32)

    rw1w_h = np.ascontiguousarray(
        (rw1_w / MEAN_N)[qof].reshape(CB, 128, C // 4).transpose(1, 0, 2)
    ).astype(BF16_NP)
    rw1b_h = np.ascontiguousarray(rw1_b[:, None]).astype(f32)

    # rw2 columns: stream-0 logits at cols [0, C), stream-1 at [C, 2C), both
    # in the interleaved row order — device M-blocks 0..3 are stream 0 and
    # 4..7 stream 1.
    rw2p = np.zeros((128, 2 * C), f32)
    rw2p[:, 0:C] = rw2_w[:, 2 * qof]
    rw2p[:, C:2 * C] = rw2_w[:, 2 * qof + 1]
    rw2w_h = np.ascontiguousarray(rw2p).astype(BF16_NP)
    rw2b_full = np.concatenate([rw2_b[2 * qof], rw2_b[2 * qof + 1]])
    rw2b_h = np.ascontiguousarray(rw2b_full.reshape(2 * CB, 128).T).astype(f32)

    fmask_h = np.zeros((128, CB), f32)
    fmask_h[GS_C:64] = 1.0
    fmask_h[64 + GS_C:128] = 1.0

    return dict(
        fcw=fcw_h, fcb=fcb_h, fc1w=fc1w_h, fc1b=fc1b_h, fc2w=fc2w_h,
        fc2b=fc2b_h, projw=projw_h, projb=projb_h, rw1w=rw1w_h, rw1b=rw1b_h,
        rw2w=rw2w_h, rw2b=rw2b_h, fmask=fmask_h,
    )


def _get_nc():
    if "nc" not in _CACHE:
        _CACHE["nc"] = build_nc()
    return _CACHE["nc"]


def run(inputs, trace=False, trace_kwargs=None):
    """Run the SPMD kernel; returns (full_output, BassKernelResults)."""
    x = np.asarray(inputs["x"], np.float32)
    shared = _prep_weights(
        np.asarray(inputs["fc_w"], np.float32), np.asarray(inputs["fc_b"], np.float32),
        np.asarray(inputs["fc1_w"], np.float32), np.asarray(inputs["fc1_b"], np.float32),
        np.asarray(inputs["fc2_w"], np.float32), np.asarray(inputs["fc2_b"], np.float32),
        np.asarray(inputs["rw1_w"], np.float32), np.asarray(inputs["rw1_b"], np.float32),
        np.asarray(inputs["rw2_w"], np.float32), np.asarray(inputs["rw2_b"], np.float32),
        np.asarray(inputs["proj_w"], np.float32), np.asarray(inputs["proj_b"], np.float32),
    )

    xf = x.reshape(B * T, HWTOK, C)
    in_maps = []
    for c in range(NCORES):
        sh = xf[NF * c:NF * (c + 1)]                      # [NF, 3136, 512]
        xt = sh.transpose(0, 2, 1).reshape(NF, CCB, 128, HWTOK)
        xt = np.ascontiguousarray(xt.transpose(0, 2, 1, 3)).astype(BF16_NP)
        m = dict(shared)
        m["xT"] = xt
        in_maps.append(m)

    nc = _get_nc()
    res = run_bass_kernel_spmd(
        nc, in_maps, list(range(NCORES)),
        trace=trace, **(dict(trace_kwargs=trace_kwargs) if trace_kwargs else {}),
    )

    out = np.empty((B * T, HWTOK, C), np.float32)
    for c in range(NCORES):
        out[NF * c:NF * (c + 1)] = np.asarray(
            res.results[c]["out"], dtype=np.float32
        )
    return out.reshape(B, T, H, W, C), res


def kernel(**inputs) -> np.ndarray:
    full, _ = run(inputs, trace=False)
    return full


# revision 48
# speedup vs baseline: 1.0080x; 1.0080x over previous
"""Trainium2 Bass kernel for nn_Mlp_cnn_shift (dense CNN MLP with 3x3 patch-shift
and a softmax-gated mix of two branches).

Strategy
--------
Data-parallel over the 16 (B,T) frames: each of the 8 NeuronCores processes 2
frames end-to-end.  All activations are kept channel-major ([C, tokens]) so the
channel contraction of every matmul has K on partitions, and `x` is
pre-transposed/cast on the host so no on-device transpose is needed.

Patch-shift handling:
 * forward shift (on xh, HID=1024): xh is stored in a zero-padded token layout
   (row pitch 57 = 56 cols + 1 zero pad col, 58-token zero guards per frame)
   and in 9 channel groups of 114 padded to 128 partitions each (host-permuted
   fc_w columns / fc1_w+fc2_w rows).  Every (dh,dw) roll then becomes a pure
   token offset in the fc1 matmul's rhs access pattern, with the zero padding
   reproducing the reference's zero-fill boundary exactly.
 * inverse shift (on gelu(y), C=512): y's channels live in an INTERLEAVED-512
   row permutation: the 8 main shift groups (57 channels each) sit at rows
   64*g .. 64*g+57, and the 9th group's 56 channels exactly fill the 7 spare
   rows of each 64-row half (rows 64*hb+57 .. 64*hb+64).  512 = 4 dense
   128-row blocks, no pad rows at all — fc1/fc2/proj all contract over
   exactly 4 K-blocks.  The gelu PSUM evacuation applies the inverse shift by
   writing each group's rows at its shifted, edge-clipped token positions
   (two 57-row writes plus two 7-row g8 fragments per block; the hardware
   accepts non-32-aligned partition bases for ScalarE/DVE access patterns).
   The w branch needs no shift, so its evacuation stays one full-block write.

Instead of w, the kernel spills d = h - w (computed per row-group on the DVE,
one row-group behind the evacuations); the post-gate combine is then just
gated = h + (a0-1)*d, two cheap DVE ops per chunk.

The only cross-core coupling is the global (T,H,W) mean feeding the softmax
gate, done as per-batch subgroup AllReduces ([0-3] and [4-7]), one per frame:
the first overlaps frame-1 compute; the second's window (collective latency
plus cross-core launch skew) is bridged by a budget of warm matmuls that
keeps the PE activity monitor from re-throttling the clock.

bf16 matmuls with f32 PSUM accumulation; output stored bf16, upcast on host.
"""

import os
import sys

for _p in ("/opt/trn_rl_repo",):
    if os.path.isdir(_p) and _p not in sys.path:
        sys.path.append(_p)

import numpy as np
import ml_dtypes

import concourse.bass as bass  # noqa: F401
import concourse.mybir as mybir
import concourse.tile as tile
from concourse import bacc
from concourse.bass_utils import run_bass_kernel_spmd

# ---------------------------------------------------------------- constants
SHIFTS = [(1, 1), (1, 0), (1, -1), (0, 1), (0, 0), (0, -1), (-1, 1), (-1, 0), (-1, -1)]
NG = 9
B, T, H, W, C = 2, 8, 56, 56, 512
HID = 1024
NCORES = 8
NF = (B * T) // NCORES          # frames per core = 2
HWTOK = H * W                   # 3136 tokens per frame
RP = W + 1                      # padded row pitch = 57
GUARD = RP + 1                  # 58 zero tokens on each end
FRPAD = RP * H                  # 3192
XHSPAN = GUARD + FRPAD + GUARD  # 3308
RG = 7                          # row groups per frame
RGR = H // RG                   # 8 rows per group
RGT = RGR * W                   # 448 valid tokens per row group
RGP = RGR * RP                  # 456 padded tokens per row group
GS_HID = 114                    # hid shift-group size (9*114 = 1026 >= 1024)
GS_C = 57                       # C shift-group size (8 main groups)
G8N = C - 8 * GS_C              # 56 channels in the 9th group
CB = C // 128                   # 4 dense C row-blocks (interleaved layout)
CCB = C // 128                  # 4
MEAN_N = float(T * H * W)
WARM_MMS = 130                  # AllReduce-window bridge matmuls
HEAD_MMS = 20                   # kernel-head HAM warmup matmuls
DCK = 256                       # output-phase token chunk

F32 = mybir.dt.float32
BF16 = mybir.dt.bfloat16
BF16_NP = ml_dtypes.bfloat16

_CACHE = {}


def _qof():
    """Interleaved-512 layout: padded row q -> real channel (a permutation).

    Main group g (57 ch) at rows 64g..64g+57; g8 channel i at row
    64*(i//7) + 57 + (i%7).
    """
    q = np.full((C,), -1, np.int64)
    for g in range(8):
        q[64 * g:64 * g + GS_C] = np.arange(GS_C * g, GS_C * (g + 1))
    for i in range(G8N):
        q[64 * (i // 7) + GS_C + (i % 7)] = 8 * GS_C + i
    assert (np.sort(q) == np.arange(C)).all()
    return q


# ---------------------------------------------------------------- device kernel
def build_nc():
    nc = bacc.Bacc("TRN2", target_bir_lowering=False, debug=False, num_devices=NCORES)

    dp = nc.declare_dram_parameter
    xT = dp("xT", [NF, 128, CCB, HWTOK], BF16, isOutput=False)
    fcw = dp("fcw", [128, CCB, NG * 128], BF16, isOutput=False)
    fcb = dp("fcb", [128, NG], F32, isOutput=False)
    fc1w = dp("fc1w", [128, NG, C], BF16, isOutput=False)
    fc1b = dp("fc1b", [128, CB], F32, isOutput=False)
    fc2w = dp("fc2w", [128, NG, C], BF16, isOutput=False)
    fc2b = dp("fc2b", [128, CB], F32, isOutput=False)
    projw = dp("projw", [128, CB, C], BF16, isOutput=False)
    projb = dp("projb", [128, C], F32, isOutput=False)
    rw1w = dp("rw1w", [128, CB, 128], BF16, isOutput=False)
    rw1b = dp("rw1b", [128, 1], F32, isOutput=False)
    rw2w = dp("rw2w", [128, 2 * C], BF16, isOutput=False)
    rw2b = dp("rw2b", [128, 2 * CB], F32, isOutput=False)
    fmask = dp("fmask", [128, CB], F32, isOutput=False)
    out_d = dp("out", [NF, HWTOK, C], BF16, isOutput=True)

    # spill space for d = h - w of each frame + collective bounce buffers
    dsp = [nc.dram_tensor(f"dsp{f}", [128, CB, HWTOK], BF16) for f in range(NF)]
    ccin = [nc.dram_tensor(f"ccin{f}", [128, CB], F32) for f in range(NF)]
    ccout = [nc.dram_tensor(f"ccout{f}", [128, CB], F32) for f in range(NF)]

    AF = mybir.ActivationFunctionType
    ALU = mybir.AluOpType
    GROUPS = [list(range(NCORES // 2)), list(range(NCORES // 2, NCORES))]

    with tile.TileContext(nc, num_cores=NCORES) as tc:
        with (
            tc.tile_pool(name="singles", bufs=1) as singles,
            tc.tile_pool(name="xh_pool", bufs=1) as xh_pool,
            tc.tile_pool(name="h_pool", bufs=2) as h_pool,
            tc.tile_pool(name="w_pool", bufs=3) as w_pool,
            tc.tile_pool(name="xt_pool", bufs=2) as xt_pool,
            tc.tile_pool(name="ostage", bufs=6) as ostage,
            tc.tile_pool(name="dstream", bufs=5) as dstream,
            tc.tile_pool(name="small", bufs=1) as small,
            tc.tile_pool(name="mmpsum", bufs=8, space="PSUM") as mmpsum,
        ):
            # ---- HAM warm-up: junk matmuls on a scratch tile, no DMA deps,
            # so the PE is already at full clock when the first real matmul
            # arrives (results discarded).
            jt = singles.tile([128, 640], BF16, name="jt")
            nc.vector.memset(jt[:], 0.0)
            for wi in range(HEAD_MMS):
                wp = mmpsum.tile([128, 512], F32, tag="mm", name=f"hw{wi}")
                nc.tensor.matmul(
                    wp[:, :512], lhsT=jt[:, 0:128], rhs=jt[:, 128:640],
                    start=True, stop=True,
                )

            # ---- load weights (resident for the whole kernel)
            def load(name, shape, dtype, src):
                t = singles.tile(shape, dtype, name=name)
                nc.sync.dma_start(out=t, in_=src[:])
                return t

            # fc weights load per k-slice so the first matmul can start after
            # ~1/4 of the transfer; the rest of the weights load while frame
            # 0's fc pass runs.
            fcw_s = singles.tile([128, CCB, NG * 128], BF16, name="fcw_s")
            for k in range(CCB):
                nc.sync.dma_start(out=fcw_s[:, k, :], in_=fcw[:, k, :])
            fcb_s = load("fcb_s", [128, NG], F32, fcb)
            _rest = {}

            def load_rest():
                _rest["fc1w_s"] = load("fc1w_s", [128, NG, C], BF16, fc1w)
                _rest["fc1b_s"] = load("fc1b_s", [128, CB], F32, fc1b)
                _rest["fc2w_s"] = load("fc2w_s", [128, NG, C], BF16, fc2w)
                _rest["fc2b_s"] = load("fc2b_s", [128, CB], F32, fc2b)
                _rest["projw_s"] = load("projw_s", [128, CB, C], BF16, projw)
                _rest["projb_s"] = load("projb_s", [128, C], F32, projb)
                _rest["rw1w_s"] = load("rw1w_s", [128, CB, 128], BF16, rw1w)
                _rest["rw1b_s"] = load("rw1b_s", [128, 1], F32, rw1b)
                _rest["rw2w_s"] = load("rw2w_s", [128, 2 * C], BF16, rw2w)
                _rest["rw2b_s"] = load("rw2b_s", [128, 2 * CB], F32, rw2b)
                _rest["fm_s"] = load("fm_s", [128, CB], F32, fmask)

            a0_s = singles.tile([128, CB], F32)   # gate for the h branch
            a1_s = singles.tile([128, CB], F32)   # a0 - 1

            # xh, padded token layout, persistent across frames.
            xh = xh_pool.tile([128, NG, XHSPAN], BF16)
            nc.vector.memset(xh[:, :, :GUARD], 0.0)
            nc.vector.memset(xh[:, :, GUARD + FRPAD:], 0.0)
            xh_rows = xh[:, :, GUARD:GUARD + FRPAD].rearrange(
                "p g (r c) -> p g r c", c=RP
            )
            nc.vector.memset(xh_rows[:, :, :, W:], 0.0)

            hw_tiles = []

            def shifted_rhs(g, rg):
                off = -(SHIFTS[g][0] * RP + SHIFTS[g][1])
                s0 = GUARD + rg * RGP + off
                return xh[:, g, s0:s0 + RGP].rearrange(
                    "p (r c) -> p r c", c=RP
                )[:, :, :W]

            for f in range(NF):
                # ---------------- A: xh = gelu(x @ fc_w + fc_b), group-blocked
                for rg in range(RG):
                    xt_t = xt_pool.tile([128, CCB, RGT], BF16, tag="xt")
                    nc.sync.dma_start(
                        out=xt_t, in_=xT[f, :, :, rg * RGT:(rg + 1) * RGT]
                    )
                    for mb in range(NG):
                        ps = mmpsum.tile([128, 512], F32, tag="mm")
                        for k in range(CCB):
                            nc.tensor.matmul(
                                ps[:, :RGT],
                                lhsT=fcw_s[:, k, mb * 128:(mb + 1) * 128],
                                rhs=xt_t[:, k, :],
                                start=(k == 0),
                                stop=(k == CCB - 1),
                            )
                        dst = xh[
                            :, mb, GUARD + rg * RGP:GUARD + (rg + 1) * RGP
                        ].rearrange("p (r c) -> p r c", c=RP)[:, :, :W]
                        src = ps[:, :RGT].rearrange("p (r c) -> p r c", c=W)
                        nc.scalar.activation(
                            out=dst, in_=src, func=AF.Gelu,
                            bias=fcb_s[:, mb:mb + 1],
                        )

                if f == 0:
                    # frame-0 fc pass is in flight; now bring in the rest
                    load_rest()
                    fc1w_s = _rest["fc1w_s"]; fc1b_s = _rest["fc1b_s"]
                    fc2w_s = _rest["fc2w_s"]; fc2b_s = _rest["fc2b_s"]
                    projw_s = _rest["projw_s"]; projb_s = _rest["projb_s"]
                    rw1w_s = _rest["rw1w_s"]; rw1b_s = _rest["rw1b_s"]
                    rw2w_s = _rest["rw2w_s"]; rw2b_s = _rest["rw2b_s"]
                    fm_s = _rest["fm_s"]

                # ---------------- C: h = invshift(gelu(shift(xh) @ fc1_w + b))
                h_t = h_pool.tile([128, CB, HWTOK], BF16, tag="h")
                nc.gpsimd.memset(h_t[:], 0.0)
                h4 = h_t.rearrange("p c (i j) -> p c i j", j=W)
                wsum_st = small.tile([128, CB, RG], F32, tag=f"wsst{f}")

                def h_evac(ps, rg, mb):
                    """Inverse-shift evacuation of one 128-row block: two
                    57-row main-group writes, plus the two 7-row g8 fragments
                    via a 32-aligned scratch activation (compute-engine APs
                    must start at 32-aligned partitions) whose g8 rows are
                    then DMA-copied into place.  No accum_out — the gate sum
                    comes from one DVE reduce over the finished h tile."""
                    ps3 = ps[:, :RGT].rearrange("p (r c) -> p r c", c=W)
                    for half in range(2):
                        p0 = half * 64
                        g = 2 * mb + half
                        sh, sw = SHIFTS[g]
                        i0 = max(0, 8 * rg - sh)
                        i1 = min(H, 8 * rg + 8 - sh)
                        j0, j1 = max(0, -sw), min(W, W - sw)
                        nc.scalar.activation(
                            out=h4[p0:p0 + GS_C, mb, i0:i1, j0:j1],
                            in_=ps3[
                                p0:p0 + GS_C,
                                i0 + sh - 8 * rg:i1 + sh - 8 * rg,
                                j0 + sw:j1 + sw,
                            ],
                            func=AF.Gelu,
                            bias=fc1b_s[p0:p0 + GS_C, mb:mb + 1],
                        )
                    # g8 fragments via 32-row scratch activations (a pattern
                    # starting at partition 32k may span at most 32
                    # partitions, so one per half)
                    sh, sw = SHIFTS[8]
                    i0 = max(0, 8 * rg - sh)
                    i1 = min(H, 8 * rg + 8 - sh)
                    j0, j1 = max(0, -sw), min(W, W - sw)
                    scr = w_pool.tile([128, RGR + 1, W], BF16, tag="scr")
                    for half in range(2):
                        q0 = half * 64 + 32
                        nc.scalar.activation(
                            out=scr[q0:q0 + 32, 0:i1 - i0, j0:j1],
                            in_=ps3[
                                q0:q0 + 32,
                                i0 + sh - 8 * rg:i1 + sh - 8 * rg,
                                j0 + sw:j1 + sw,
                            ],
                            func=AF.Gelu,
                            bias=fc1b_s[q0:q0 + 32, mb:mb + 1],
                        )
                        f0 = half * 64 + GS_C
                        nc.sync.dma_start(
                            out=h4[f0:f0 + 7, mb, i0:i1, j0:j1],
                            in_=scr[f0:f0 + 7, 0:i1 - i0, j0:j1],
                        )

                # ---------------- C+B merged per row-group: fc1 (h, shifted
                # evacuation) and fc2 (w) matmuls interleaved so the ScalarE
                # evacuation load stays well under the PE matmul rate; then
                # d = h - w one row-group behind (spilled for the out phase).
                dsum_st = small.tile([128, CB, RG], F32, tag=f"dsst{f}")
                prev_w = [None] * RG

                def d_spill(rg):
                    w_prev = prev_w[rg]
                    nc.vector.tensor_tensor(
                        w_prev[:],
                        h_t[:, :, rg * RGT:(rg + 1) * RGT],
                        w_prev[:],
                        ALU.subtract,
                    )
                    nc.sync.dma_start(
                        out=dsp[f][:, :, rg * RGT:(rg + 1) * RGT], in_=w_prev[:]
                    )
                    # gate-sum bookkeeping: sum(h) = sum(d) + sum(w)
                    nc.vector.tensor_reduce(
                        out=dsum_st[:, :, rg:rg + 1], in_=w_prev[:],
                        axis=mybir.AxisListType.X, op=ALU.add,
                    )

                for rg in range(RG):
                    w_rg = w_pool.tile([128, CB, RGT], BF16, tag="wrg")
                    rhs_pl = xh[:, :, GUARD + rg * RGP:GUARD + (rg + 1) * RGP
                                ].rearrange("p g (r c) -> p g r c", c=RP)[:, :, :, :W]
                    for mb in range(CB):
                        ps = mmpsum.tile([128, 512], F32, tag="mm")
                        for g in range(NG):
                            nc.tensor.matmul(
                                ps[:, :RGT],
                                lhsT=fc1w_s[:, g, mb * 128:(mb + 1) * 128],
                                rhs=shifted_rhs(g, rg),
                                start=(g == 0),
                                stop=(g == NG - 1),
                            )
                        h_evac(ps, rg, mb)
                        ps = mmpsum.tile([128, 512], F32, tag="mm")
                        for g in range(NG):
                            nc.tensor.matmul(
                                ps[:, :RGT],
                                lhsT=fc2w_s[:, g, mb * 128:(mb + 1) * 128],
                                rhs=rhs_pl[:, g],
                                start=(g == 0),
                                stop=(g == NG - 1),
                            )
                        dst = w_rg[:, mb, :].rearrange("p (r c) -> p r c", c=W)
                        srcp = ps[:, :RGT].rearrange("p (r c) -> p r c", c=W)
                        nc.scalar.activation(
                            out=dst, in_=srcp, func=AF.Gelu,
                            bias=fc2b_s[:, mb:mb + 1],
                        )
                    nc.vector.tensor_reduce(
                        out=wsum_st[:, :, rg:rg + 1], in_=w_rg[:],
                        axis=mybir.AxisListType.X, op=ALU.add,
                    )

                    prev_w[rg] = w_rg
                    if rg >= 1:
                        d_spill(rg - 1)
                d_spill(RG - 1)

                # ---------------- per-frame gate partial sums + AllReduce
                # part = sum(h) + sum(w) = sum(d) + 2*sum(w)
                ws = small.tile([128, CB], F32, tag=f"ws{f}")
                nc.vector.tensor_reduce(
                    out=ws, in_=wsum_st[:], axis=mybir.AxisListType.X, op=ALU.add
                )
                dsm = small.tile([128, CB], F32, tag=f"dsm{f}")
                nc.vector.tensor_reduce(
                    out=dsm, in_=dsum_st[:], axis=mybir.AxisListType.X, op=ALU.add
                )
                part = small.tile([128, CB], F32, tag=f"part{f}")
                nc.vector.tensor_scalar_mul(part, ws, 2.0)
                nc.vector.tensor_tensor(part, part, dsm, ALU.add)
                nc.sync.dma_start(out=ccin[f][:], in_=part)
                nc.gpsimd.collective_compute(
                    "AllReduce",
                    ALU.add,
                    replica_groups=GROUPS,
                    ins=[ccin[f][:]],
                    outs=[ccout[f][:]],
                )

                hw_tiles.append(h_t)

            # bridge the second AllReduce's latency window (collective floor
            # + cross-core launch skew) with junk matmuls so the PE activity
            # monitor keeps the full clock.
            for wi in range(WARM_MMS):
                wp = mmpsum.tile([128, 512], F32, tag="mm", name=f"warm{wi}")
                nc.tensor.matmul(
                    wp[:, :512],
                    lhsT=fcw_s[:, 0, 0:128],
                    rhs=fcw_s[:, 1, 0:512],
                    start=True,
                    stop=True,
                )

            # ---------------- combine the two AllReduce results -> z
            acc = []
            for f in range(NF):
                za = small.tile([128, CB], F32, tag=f"za{f}")
                nc.sync.dma_start(out=za, in_=ccout[f][:])
                acc.append(za)
            zsum = small.tile([128, CB], F32, tag="zsum")
            nc.vector.tensor_tensor(zsum, acc[0], acc[1], ALU.add)
            zbf = small.tile([128, CB], BF16, tag="zbf")
            nc.vector.tensor_copy(out=zbf, in_=zsum)

            # ---------------- gate: a = softmax over the 2 streams
            # (1/MEAN_N is folded into rw1w on the host)
            psg = mmpsum.tile([128, 512], F32, tag="mm", name="psg")[:, :1]
            for k in range(CB):
                nc.tensor.matmul(
                    psg,
                    lhsT=rw1w_s[:, k, :],
                    rhs=zbf[:, k:k + 1],
                    start=(k == 0),
                    stop=(k == CB - 1),
                )
            gv = small.tile([128, 1], BF16, tag="gv")
            nc.scalar.activation(out=gv, in_=psg, func=AF.Gelu, bias=rw1b_s[:, 0:1])
            psu = mmpsum.tile([128, 512], F32, tag="mm", name="psu")[:, :2 * CB]
            for m in range(2 * CB):
                nc.tensor.matmul(
                    psu[:, m:m + 1],
                    lhsT=rw2w_s[:, m * 128:(m + 1) * 128],
                    rhs=gv,
                    start=True,
                    stop=True,
                )
            # softmax over 2 streams == sigmoid of the logit difference:
            # a0 = e0/(e0+e1) = sigmoid(l0 - l1), a1 = a0 - 1
            uv = small.tile([128, 2 * CB], F32, tag="uv")
            nc.vector.tensor_tensor(uv, psu, rw2b_s, ALU.add)
            ldif = small.tile([128, CB], F32, tag="ldif")
            nc.vector.tensor_tensor(ldif, uv[:, 0:CB], uv[:, CB:2 * CB],
                                    ALU.subtract)
            nc.scalar.activation(out=a0_s, in_=ldif, func=AF.Sigmoid)
            nc.vector.tensor_scalar_sub(a1_s, a0_s, 1.0)

            # ---------------- D: out = (h + (a0-1)*d) @ proj_w + proj_b
            def emit_out(pp, fidx, t0, M):
                ot = ostage.tile([128, C], BF16, tag="ot")
                nc.vector.tensor_tensor(ot[:M], pp[:M, :C], projb_s[:M], ALU.add)
                nc.sync.dma_start(out=out_d[fidx, t0:t0 + M, :], in_=ot[:M])

            # h is resident for both frames; stream each frame's d back in
            # DCK-token chunks, gate into the d tile, then project.  The two
            # frames' chunks are interleaved so their independent
            # DMA->gate->matmul chains hide each other's latency.
            for ck0 in range(0, HWTOK, DCK):
                for fidx in (0, 1):
                    h_t = hw_tiles[fidx]
                    CK = min(DCK, HWTOK - ck0)
                    dc = dstream.tile([128, CB, DCK], BF16, tag="wc")
                    nc.sync.dma_start(
                        out=dc[:, :, :CK], in_=dsp[fidx][:, :, ck0:ck0 + CK]
                    )
                    # gating scale split across ScalarE and DVE so neither
                    # paces the chunk pipeline
                    for kb in range(2):
                        nc.scalar.activation(
                            out=dc[:, kb, :CK], in_=dc[:, kb, :CK],
                            func=AF.Copy, scale=a1_s[:, kb:kb + 1],
                        )
                    for kb in range(2, CB):
                        nc.vector.tensor_scalar_mul(
                            dc[:, kb, :CK], dc[:, kb, :CK], a1_s[:, kb:kb + 1]
                        )
                    nc.vector.tensor_tensor(
                        dc[:, :, :CK],
                        h_t[:, :, ck0:ck0 + CK],
                        dc[:, :, :CK],
                        ALU.add,
                    )
                    m0 = 0
                    while m0 < CK:
                        M = min(128, CK - m0)
                        pp = mmpsum.tile([128, 512], F32, tag="mm")
                        for kb in range(CB):
                            nc.tensor.matmul(
                                pp[:M, :C],
                                lhsT=dc[:, kb, m0:m0 + M],
                                rhs=projw_s[:, kb, :],
                                start=(kb == 0),
                                stop=(kb == CB - 1),
                            )
                        emit_out(pp, fidx, ck0 + m0, M)
                        m0 += M

    nc.compile()
    return nc


# ---------------------------------------------------------------- host side
def _prep_weights(fc_w, fc_b, fc1_w, fc1_b, fc2_w, fc2_b,
                  rw1_w, rw1_b, rw2_w, rw2_b, proj_w, proj_b):
    f32 = np.float32
    qof = _qof()

    # fc: columns permuted into 9 HID-groups of 114 (112 for g=8), pad to 128
    fcwp = np.zeros((C, NG * 128), f32)
    fcbp = np.zeros((NG * 128,), f32)
    for g in range(NG):
        n = min(GS_HID * (g + 1), HID) - GS_HID * g
        fcwp[:, 128 * g:128 * g + n] = fc_w[:, GS_HID * g:GS_HID * g + n]
        fcbp[128 * g:128 * g + n] = fc_b[GS_HID * g:GS_HID * g + n]
    fcw_h = np.ascontiguousarray(
        fcwp.reshape(CCB, 128, NG * 128).transpose(1, 0, 2)
    ).astype(BF16_NP)
    fcb_h = np.ascontiguousarray(fcbp.reshape(NG, 128).T).astype(f32)

    def hid_rows_grouped(wm):  # [HID, C] -> [128, NG, C] padded group rows
        wp = np.zeros((NG * 128, wm.shape[1]), f32)
        for g in range(NG):
            n = min(GS_HID * (g + 1), HID) - GS_HID * g
            wp[128 * g:128 * g + n] = wm[GS_HID * g:GS_HID * g + n]
        return np.ascontiguousarray(
            wp.reshape(NG, 128, wm.shape[1]).transpose(1, 0, 2)
        ).astype(BF16_NP)

    # fc1/fc2 columns, proj/rw1 rows, biases: interleaved-512 permutation
    fc1w_h = hid_rows_grouped(fc1_w[:, qof])
    fc2w_h = hid_rows_grouped(fc2_w[:, qof])
    fc1b_h = np.ascontiguousarray(fc1_b[qof].reshape(CB, 128).T).astype(f32)
    fc2b_h = np.ascontiguousarray(fc2_b[qof].reshape(CB, 128).T).astype(f32)

    projw_h = np.ascontiguousarray(
        proj_w[qof].reshape(CB, 128, C).transpose(1, 0, 2)
    ).astype(BF16_NP)
    projb_h = np.ascontiguousarray(
        np.broadcast_to(proj_b[None, :], (128, C))
    ).astype(f32)

    rw1w_h = np.ascontiguousarray(
        (rw1_w / MEAN_N)[qof].reshape(CB, 128, C // 4).transpose(1, 0, 2)
    ).astype(BF16_NP)
    rw1b_h = np.ascontiguousarray(rw1_b[:, None]).astype(f32)

    # rw2 columns: stream-0 logits at cols [0, C), stream-1 at [C, 2C), both
    # in the interleaved row order — device M-blocks 0..3 are stream 0 and
    # 4..7 stream 1.
    rw2p = np.zeros((128, 2 * C), f32)
    rw2p[:, 0:C] = rw2_w[:, 2 * qof]
    rw2p[:, C:2 * C] = rw2_w[:, 2 * qof + 1]
    rw2w_h = np.ascontiguousarray(rw2p).astype(BF16_NP)
    rw2b_full = np.concatenate([rw2_b[2 * qof], rw2_b[2 * qof + 1]])
    rw2b_h = np.ascontiguousarray(rw2b_full.reshape(2 * CB, 128).T).astype(f32)

    fmask_h = np.zeros((128, CB), f32)
    fmask_h[GS_C:64] = 1.0
    fmask_h[64 + GS_C:128] = 1.0

    return dict(
        fcw=fcw_h, fcb=fcb_h, fc1w=fc1w_h, fc1b=fc1b_h, fc2w=fc2w_h,
        fc2b=fc2b_h, projw=projw_h, projb=projb_h, rw1w=rw1w_h, rw1b=rw1b_h,
        rw2w=rw2w_h, rw2b=rw2b_h, fmask=fmask_h,
    )


def _get_nc():
    if "nc" not in _CACHE:
        _CACHE["nc"] = build_nc()
    return _CACHE["nc"]


def run(inputs, trace=False, trace_kwargs=None):
    """Run the SPMD kernel; returns (full_output, BassKernelResults)."""
    x = np.asarray(inputs["x"], np.float32)
    shared = _prep_weights(
        np.asarray(inputs["fc_w"], np.float32), np.asarray(inputs["fc_b"], np.float32),
        np.asarray(inputs["fc1_w"], np.float32), np.asarray(inputs["fc1_b"], np.float32),
        np.asarray(inputs["fc2_w"], np.float32), np.asarray(inputs["fc2_b"], np.float32),
        np.asarray(inputs["rw1_w"], np.float32), np.asarray(inputs["rw1_b"], np.float32),
        np.asarray(inputs["rw2_w"], np.float32), np.asarray(inputs["rw2_b"], np.float32),
        np.asarray(inputs["proj_w"], np.float32), np.asarray(inputs["proj_b"], np.float32),
    )

    xf = x.reshape(B * T, HWTOK, C)
    in_maps = []
    for c in range(NCORES):
        sh = xf[NF * c:NF * (c + 1)]                      # [NF, 3136, 512]
        xt = sh.transpose(0, 2, 1).reshape(NF, CCB, 128, HWTOK)
        xt = np.ascontiguousarray(xt.transpose(0, 2, 1, 3)).astype(BF16_NP)
        m = dict(shared)
        m["xT"] = xt
        in_maps.append(m)

    nc = _get_nc()
    res = run_bass_kernel_spmd(
        nc, in_maps, list(range(NCORES)),
        trace=trace, **(dict(trace_kwargs=trace_kwargs) if trace_kwargs else {}),
    )

    out = np.empty((B * T, HWTOK, C), np.float32)
    for c in range(NCORES):
        out[NF * c:NF * (c + 1)] = np.asarray(
            res.results[c]["out"], dtype=np.float32
        )
    return out.reshape(B, T, H, W, C), res


def kernel(**inputs) -> np.ndarray:
    full, _ = run(inputs, trace=False)
    return full
